# revision 36
# baseline (speedup 1.0000x reference)
"""Ensemble low-bit-decoded 3x3 conv2d, data-parallel over 8 TRN2 NeuronCores.

Problem (hardcoded): x (16, 64, 160, 160) f32. 4 ensemble members; image b uses
ensemble n = b % 4. Weights (64, 64, 3, 3) per ensemble are decoded from tiny
U/V/scale/biasq params:
    w = scale_n * (sigmoid(clip(U_n*V_0)) + 2*sigmoid(clip(U_n*V_1)) - biasq_n - 4)
then out[b] = conv2d(x[b], w_{b%4}, pad=1) + bias_{b%4}.

Sharding: core j gets images (2j, 2j+1); weights decoded host-side (tiny) and
shipped as ready fp16 lhsT tiles.

Kernel strategy per image (pure conv on device, memory-roofline oriented):
  Host packs the image into a parity SBUF layout, fp16, zero pads baked in:
  partition ci<64 = channel ci of even padded row r'=2s, 64+ci = odd r'=2s+1,
  free offset s*161 + c (col 0 = shared left/right pad). All device DMAs are
  therefore large fully-contiguous-per-partition transfers. A matmul with
  K=128 = (2 rows x 64 cin), M=128 = (2 out rows x 64 cout) covers up to 4 conv
  taps; 6 matmuls (2 row-phases x 3 kw shifts) accumulate a PSUM supertile of
  2-3 output row-pairs (F<=482), covering all 9 taps of the 3x3 stencil.
  Output is written fp16 in the same parity layout and unpacked on host.
"""

import os

import numpy as np

import concourse.bass as bass
import concourse.mybir as mybir
import concourse.tile as tile
from concourse import bacc

N = 4
CIN = 64
COUT = 64
KS = 3
NB = 2  # weight bits
H = 160
W = 160
N_CORES = 8
N_IMG = 2  # images per core

PW = W + 1  # pair stride in the shared-pad layout
NPAIR = (H + 2) // 2  # 81 padded row-pairs
NPX = NPAIR * PW + 1  # free elements per image per partition (13042)
OUT_PAIRS = H // 2  # 80

F32 = mybir.dt.float32
F16 = mybir.dt.float16


def build_nc(n_img=N_IMG, band_out_pairs=20, st_pairs=3, n_in_chunks=8):
    """Build the single-core Bass program (SPMD: all cores run this)."""
    assert OUT_PAIRS % band_out_pairs == 0
    n_bands = OUT_PAIRS // band_out_pairs

    nc = bacc.Bacc("TRN2", target_bir_lowering=False, num_swdge_queues=4)

    x3 = nc.dram_tensor("x3", (n_img, 128, NPX), F16, kind="ExternalInput")
    wl = nc.dram_tensor("wl", (128, n_img * 6, 128), F16, kind="ExternalInput")
    bv = nc.dram_tensor("bv", (128, n_img), F32, kind="ExternalInput")
    out3 = nc.dram_tensor(
        "out3", (n_img, 128, OUT_PAIRS, W), F16, kind="ExternalOutput"
    )

    AF = mybir.ActivationFunctionType

    # supertile split of each band; the last band tapers to a 1-pair final
    # supertile so the very last ACT + store are tiny
    def split(ks):
        o, offs = 0, []
        for k in ks:
            offs.append(o)
            o += k
        assert o == band_out_pairs
        return list(zip(ks, offs))

    sts = []
    rem = band_out_pairs
    while rem > 0:
        k = min(st_pairs, rem)
        sts.append(k)
        rem -= k
    st_norm = split(sts)
    st_last = split([3, 3, 3, 3, 3, 2, 2, 1])
    # last band's output chunks: (lo, hi, after-supertile-index); tapered so
    # the post-compute tail is just a 1-pair store
    oc_last = [(0, 9, 3), (9, 15, 4), (15, 17, 5), (17, 19, 6), (19, 20, 7)]

    with tile.TileContext(nc) as tc:
        with (
            tc.tile_pool(name="wts", bufs=1) as wpool,
            tc.tile_pool(name="xbuf", bufs=1) as xpool,
            tc.tile_pool(name="obuf", bufs=4) as opool,
            tc.tile_pool(name="psum", bufs=8, space="PSUM") as pspool,
        ):
            # HAM warmup: the PE clock sits at 1.2 GHz until ~3.4us of
            # sustained activity (a gap resets the free-running window). A
            # continuous train of small matmuls paces the PE from preamble
            # end (~7.5us) until the first real data lands (~10.5us): HAM
            # flips to full clock mid-train and the real stream starts warm.
            # The count is tuned to the data arrival — too few leaves a gap
            # (stream runs cold), too many delays the real stream.
            # memset on gpsimd: its preamble ends ~1us before vector's, so
            # the train starts sooner and the HAM flip lands earlier
            scr = wpool.tile([128, 512], F16, tag="scr")
            nc.gpsimd.memset(scr[:], 0.0)
            wps = pspool.tile([128, 256], F32, tag="ps", name="ps")
            for _ in range(20):
                nc.tensor.matmul(
                    wps[:], scr[:, 0:128], scr[:, 0:256], start=True, stop=True
                )

            w_sb = wpool.tile([128, n_img * 6, 128], F16, tag="w")
            bv_sb = wpool.tile([128, n_img], F32, tag="bv")
            xt = xpool.tile([128, n_img, NPX], F16, tag="x")

            # ALL loads ride ONE sync-ring FIFO in need-order: weights first,
            # then chunks in consumption order. The deep in-order supply
            # queue stays 1-2 chunks ahead of the matmul stream and cannot be
            # starved by another initiator racing ahead during the slow
            # post-idle HBM ramp (which capped aggregate rate at ~150 GB/s).
            bnds0 = [0, 4 * PW + 1, 9 * PW + 1, 14 * PW + 1, 21 * PW + 1] + [
                (31 + 10 * c) * PW + 1 for c in range(5)
            ] + [NPX]
            x0c = [(0, bnds0[c], bnds0[c + 1]) for c in range(len(bnds0) - 1)]
            cpairs = NPAIR // n_in_chunks
            bnds1 = [cpairs * c * PW for c in range(n_in_chunks)] + [NPX]
            x1c = [(1, bnds1[c], bnds1[c + 1]) for c in range(len(bnds1) - 1)]

            def xdma(eng, i, lo, hi):
                eng.dma_start(
                    out=xt[:, i, lo:hi], in_=x3[i, :, lo:hi]
                )

            # image-0 weights split phase-1/phase-2: supertile 0's first
            # three matmuls need only w0a + pairs 0-3, so the stream starts
            # ~1.5us sooner
            nc.sync.dma_start(out=w_sb[:, 0:3, :], in_=wl[:, 0:3, :])
            xdma(nc.sync, *x0c[0])
            nc.sync.dma_start(out=w_sb[:, 3:6, :], in_=wl[:, 3:6, :])
            for c in x0c[1:]:
                xdma(nc.sync, *c)
            nc.sync.dma_start(out=w_sb[:, 6:12, :], in_=wl[:, 6:12, :])
            for c in x1c:
                xdma(nc.sync, *c)
            # bias rides the otherwise-idle scalar ring (needed by first ACT)
            nc.scalar.dma_start(out=bv_sb[:], in_=bv[:, :])

            for i in range(n_img):
                for band in range(n_bands):
                    s0 = band * band_out_pairs
                    last = i == n_img - 1 and band == n_bands - 1
                    ob = opool.tile([128, band_out_pairs, W], F16, tag="ob")
                    for sti, (k, off) in enumerate(st_last if last else st_norm):
                        ps = pspool.tile([128, k * PW], F32, tag="ps", name="ps")
                        f = k * PW - 1
                        for ph in range(2):
                            for kw in range(KS):
                                widx = ph * 3 + kw
                                a = (s0 + off + ph) * PW + kw
                                nc.tensor.matmul(
                                    ps[:, 0:f],
                                    w_sb[:, i * 6 + widx, :],
                                    xt[:, i, a : a + f],
                                    start=(widx == 0),
                                    stop=(widx == 5),
                                )
                        ps3 = ps.rearrange("p (t c) -> p t c", t=k)
                        nc.scalar.activation(
                            ob[:, off : off + k, :],
                            ps3[:, :, 0:W],
                            AF.Identity,
                            bias=bv_sb[:, i : i + 1],
                            scale=1.0,
                        )
                        if last:
                            # stream out in shrinking chunks on the (by now
                            # idle) sync HWDGE ring; the final 1-pair store
                            # issues from the scalar engine right behind its
                            # own ACT, dodging sync-ring issue serialization
                            for lo, hi, after in oc_last:
                                if after == sti:
                                    eng = (
                                        nc.scalar
                                        if after == len(st_last) - 1
                                        else nc.sync
                                    )
                                    eng.dma_start(
                                        out=out3[i, :, s0 + lo : s0 + hi, :],
                                        in_=ob[:, lo:hi, :],
                                    )
                    if not last:
                        # two half-band output DMAs (SWDGE ring, decoupled
                        # from the scalar ACT stream)
                        hb = band_out_pairs // 2
                        nc.gpsimd.dma_start(
                            out=out3[i, :, s0 : s0 + hb, :], in_=ob[:, 0:hb, :]
                        )
                        nc.gpsimd.dma_start(
                            out=out3[i, :, s0 + hb : s0 + band_out_pairs, :],
                            in_=ob[:, hb:band_out_pairs, :],
                        )

    nc.compile()
    return nc


_NC_CACHE = {}


def _get_nc():
    if "nc" not in _NC_CACHE:
        _NC_CACHE["nc"] = build_nc()
    return _NC_CACHE["nc"]


def _decode_weights(U, V, twopow, scale, biasq, bias):
    """Host-side decode of the tiny weight params -> fp16 lhsT tiles.

    Returns L (N, 6, 128, 128) fp16 and bn (N, 128) f32.
    lhsT tile widx = ph*3+kw, K index q = (row parity, cin), M = (out parity j,
    cout). Phase 1 reads rhs pair m (padded rows 2m, 2m+1), phase 2 pair m+1.
    """
    theta = np.einsum("ndk,bdk->nbd", U, V)  # (N, NB, D)
    sb = 1.0 / (1.0 + np.exp(-np.clip(theta, -10.0, 10.0)))
    integer = np.einsum("nbd,b->nd", sb, twopow)
    w = scale * (integer - biasq - 2.0**NB)  # (N, D)
    w = w.reshape(N, COUT, CIN, KS, KS)
    wq = np.ascontiguousarray(w.transpose(0, 2, 1, 3, 4))  # (n, ci, co, kh, kw)

    L = np.zeros((N, 6, 128, 128), np.float16)
    for kw in range(KS):
        # phase 1: (q0 -> j0): kh0, (q1 -> j0): kh1, (q1 -> j1): kh0
        L[:, kw, 0:64, 0:64] = wq[:, :, :, 0, kw]
        L[:, kw, 64:128, 0:64] = wq[:, :, :, 1, kw]
        L[:, kw, 64:128, 64:128] = wq[:, :, :, 0, kw]
        # phase 2: (q0 -> j0): kh2, (q0 -> j1): kh1, (q1 -> j1): kh2
        L[:, 3 + kw, 0:64, 0:64] = wq[:, :, :, 2, kw]
        L[:, 3 + kw, 0:64, 64:128] = wq[:, :, :, 1, kw]
        L[:, 3 + kw, 64:128, 64:128] = wq[:, :, :, 2, kw]

    bn = bias.reshape(N, COUT).astype(np.float32)
    bn = np.concatenate([bn, bn], axis=1)  # (N, 128)
    return L, bn


def _pack_x(xb):
    """(n, 64, 160, 160) f32 -> (n, 128, NPX) f16 parity layout, pads baked."""
    n = xb.shape[0]
    P = np.zeros((n, CIN, H + 2, H + 2), np.float16)
    P[:, :, 1 : H + 1, 1 : W + 1] = xb
    ev = P[:, :, 0 : H + 2 : 2, 0 : W + 1]  # (n, 64, 81, 161) padded rows 2s
    od = P[:, :, 1 : H + 2 : 2, 0 : W + 1]  # padded rows 2s+1
    arr = np.concatenate([ev, od], axis=1).reshape(n, 128, NPAIR * PW)
    out = np.zeros((n, 128, NPX), np.float16)
    out[:, :, 0 : NPAIR * PW] = arr
    return out


LAST_RESULT = None


def _ensure_ntff_hook():
    """The container's antenv package lacks axon_hooks; synthesize it so
    run_bass_kernel_spmd(trace=True) can register the NTFF profiler."""
    import sys
    import types

    if "antenv.axon_hooks" in sys.modules:
        return True
    try:
        import antenv
        from trn_agent_boot.trn_boot import _ntff_profile_via_ctypes

        hook = _ntff_profile_via_ctypes("/opt/axon/libaxon_pjrt.so")
        mod = types.ModuleType("antenv.axon_hooks")
        mod._hook = hook
        mod.get_axon_ntff_profile_hook = lambda: mod._hook
        mod.set_axon_ntff_profile_hook = lambda h: setattr(mod, "_hook", h)
        sys.modules["antenv.axon_hooks"] = mod
        antenv.axon_hooks = mod
        return hook is not None
    except Exception as e:  # degrade to untraced run
        print(f"ntff hook setup failed: {type(e).__name__}: {e}")
        return False


def kernel(x, U, V, twopow, scale, biasq, bias):
    from concourse.bass_utils import run_bass_kernel_spmd

    global LAST_RESULT
    x = np.asarray(x, np.float32)
    L, bn = _decode_weights(
        np.asarray(U, np.float32),
        np.asarray(V, np.float32),
        np.asarray(twopow, np.float32),
        np.asarray(scale, np.float32),
        np.asarray(biasq, np.float32),
        np.asarray(bias, np.float32),
    )

    in_maps = []
    for j in range(N_CORES):
        bs = [N_IMG * j + t for t in range(N_IMG)]
        ns = [b % N for b in bs]
        wlj = np.ascontiguousarray(
            L[ns].reshape(N_IMG * 6, 128, 128).transpose(1, 0, 2)
        )  # (128, n_img*6, 128)
        bvj = np.ascontiguousarray(bn[ns].T)  # (128, n_img)
        in_maps.append(
            {
                "x3": _pack_x(x[bs]),
                "wl": wlj,
                "bv": bvj,
            }
        )

    nc = _get_nc()
    trace = bool(os.environ.get("KERNEL_TRACE"))
    if trace:
        trace = _ensure_ntff_hook()
    tmpdir = os.environ.get("KERNEL_TRACE_DIR") or None
    res = run_bass_kernel_spmd(
        nc, in_maps, list(range(N_CORES)), trace=trace, tmpdir=tmpdir
    )
    LAST_RESULT = res

    out = np.empty((16, COUT, H, W), np.float32)
    for j in range(N_CORES):
        o3 = res.results[j]["out3"].astype(np.float32)  # (n_img, 128, 80, 160)
        for i in range(N_IMG):
            b = N_IMG * j + i
            out[b, :, 0::2, :] = o3[i, 0:64]
            out[b, :, 1::2, :] = o3[i, 64:128]
    return out


# revision 37
# speedup vs baseline: 1.0082x; 1.0082x over previous
"""Ensemble low-bit-decoded 3x3 conv2d, data-parallel over 8 TRN2 NeuronCores.

Problem (hardcoded): x (16, 64, 160, 160) f32. 4 ensemble members; image b uses
ensemble n = b % 4. Weights (64, 64, 3, 3) per ensemble are decoded from tiny
U/V/scale/biasq params:
    w = scale_n * (sigmoid(clip(U_n*V_0)) + 2*sigmoid(clip(U_n*V_1)) - biasq_n - 4)
then out[b] = conv2d(x[b], w_{b%4}, pad=1) + bias_{b%4}.

Sharding: core j gets images (2j, 2j+1); weights decoded host-side (tiny) and
shipped as ready fp16 lhsT tiles.

Kernel strategy per image (pure conv on device, memory-roofline oriented):
  Host packs the image into a parity SBUF layout, fp16, zero pads baked in:
  partition ci<64 = channel ci of even padded row r'=2s, 64+ci = odd r'=2s+1,
  free offset s*161 + c (col 0 = shared left/right pad). All device DMAs are
  therefore large fully-contiguous-per-partition transfers. A matmul with
  K=128 = (2 rows x 64 cin), M=128 = (2 out rows x 64 cout) covers up to 4 conv
  taps; 6 matmuls (2 row-phases x 3 kw shifts) accumulate a PSUM supertile of
  2-3 output row-pairs (F<=482), covering all 9 taps of the 3x3 stencil.
  Output is written fp16 in the same parity layout and unpacked on host.
"""

import os

import numpy as np

import concourse.bass as bass
import concourse.mybir as mybir
import concourse.tile as tile
from concourse import bacc

N = 4
CIN = 64
COUT = 64
KS = 3
NB = 2  # weight bits
H = 160
W = 160
N_CORES = 8
N_IMG = 2  # images per core

PW = W + 1  # pair stride in the shared-pad layout
NPAIR = (H + 2) // 2  # 81 padded row-pairs
NPX = NPAIR * PW + 1  # free elements per image per partition (13042)
OUT_PAIRS = H // 2  # 80

F32 = mybir.dt.float32
F16 = mybir.dt.float16


def build_nc(n_img=N_IMG, band_out_pairs=20, st_pairs=3, n_in_chunks=8):
    """Build the single-core Bass program (SPMD: all cores run this)."""
    assert OUT_PAIRS % band_out_pairs == 0
    n_bands = OUT_PAIRS // band_out_pairs

    nc = bacc.Bacc("TRN2", target_bir_lowering=False, num_swdge_queues=4)

    x3 = nc.dram_tensor("x3", (n_img, 128, NPX), F16, kind="ExternalInput")
    wl = nc.dram_tensor("wl", (128, n_img * 6, 128), F16, kind="ExternalInput")
    bv = nc.dram_tensor("bv", (128, n_img), F32, kind="ExternalInput")
    out3 = nc.dram_tensor(
        "out3", (n_img, 128, OUT_PAIRS, W), F16, kind="ExternalOutput"
    )

    AF = mybir.ActivationFunctionType

    # supertile split of each band; the last band tapers to a 1-pair final
    # supertile so the very last ACT + store are tiny
    def split(ks):
        o, offs = 0, []
        for k in ks:
            offs.append(o)
            o += k
        assert o == band_out_pairs
        return list(zip(ks, offs))

    sts = []
    rem = band_out_pairs
    while rem > 0:
        k = min(st_pairs, rem)
        sts.append(k)
        rem -= k
    st_norm = split(sts)
    st_last = split([3, 3, 3, 3, 3, 2, 2, 1])
    # last band's output chunks: (lo, hi, after-supertile-index); tapered so
    # the post-compute tail is just a 1-pair store
    oc_last = [(0, 9, 3), (9, 15, 4), (15, 17, 5), (17, 19, 6), (19, 20, 7)]

    with tile.TileContext(nc) as tc:
        with (
            tc.tile_pool(name="wts", bufs=1) as wpool,
            tc.tile_pool(name="xbuf", bufs=1) as xpool,
            tc.tile_pool(name="obuf", bufs=4) as opool,
            tc.tile_pool(name="psum", bufs=8, space="PSUM") as pspool,
        ):
            # HAM warmup: the PE clock sits at 1.2 GHz until ~3.4us of
            # sustained activity (a gap resets the free-running window). A
            # continuous train of small matmuls paces the PE from preamble
            # end (~7.5us) until the first real data lands (~10.5us): HAM
            # flips to full clock mid-train and the real stream starts warm.
            # The count is tuned to the data arrival — too few leaves a gap
            # (stream runs cold), too many delays the real stream.
            # (memset on gpsimd — its preamble ends first; the train start
            # itself is pinned by the tensor engine's preamble at ~7.4us)
            scr = wpool.tile([128, 512], F16, tag="scr")
            nc.gpsimd.memset(scr[:], 0.0)
            wps = pspool.tile([128, 256], F32, tag="ps", name="ps")
            for _ in range(15):
                nc.tensor.matmul(
                    wps[:], scr[:, 0:128], scr[:, 0:256], start=True, stop=True
                )

            w_sb = wpool.tile([128, n_img * 6, 128], F16, tag="w")
            bv_sb = wpool.tile([128, n_img], F32, tag="bv")
            xt = xpool.tile([128, n_img, NPX], F16, tag="x")

            # ALL loads ride ONE sync-ring FIFO in need-order: weights first,
            # then chunks in consumption order. The deep in-order supply
            # queue stays 1-2 chunks ahead of the matmul stream and cannot be
            # starved by another initiator racing ahead during the slow
            # post-idle HBM ramp (which capped aggregate rate at ~150 GB/s).
            bnds0 = [0, 4 * PW + 1, 9 * PW + 1, 14 * PW + 1, 21 * PW + 1] + [
                (31 + 10 * c) * PW + 1 for c in range(5)
            ] + [NPX]
            x0c = [(0, bnds0[c], bnds0[c + 1]) for c in range(len(bnds0) - 1)]
            cpairs = NPAIR // n_in_chunks
            bnds1 = [cpairs * c * PW for c in range(n_in_chunks)] + [NPX]
            x1c = [(1, bnds1[c], bnds1[c + 1]) for c in range(len(bnds1) - 1)]

            def xdma(eng, i, lo, hi):
                eng.dma_start(
                    out=xt[:, i, lo:hi], in_=x3[i, :, lo:hi]
                )

            # image-0 weights split phase-1/phase-2: supertile 0's first
            # three matmuls need only w0a + pairs 0-3, so the stream starts
            # ~1.5us sooner
            nc.sync.dma_start(out=w_sb[:, 0:3, :], in_=wl[:, 0:3, :])
            xdma(nc.sync, *x0c[0])
            nc.sync.dma_start(out=w_sb[:, 3:6, :], in_=wl[:, 3:6, :])
            for c in x0c[1:]:
                xdma(nc.sync, *c)
            nc.sync.dma_start(out=w_sb[:, 6:12, :], in_=wl[:, 6:12, :])
            for c in x1c:
                xdma(nc.sync, *c)
            # bias rides the otherwise-idle scalar ring (needed by first ACT)
            nc.scalar.dma_start(out=bv_sb[:], in_=bv[:, :])

            for i in range(n_img):
                for band in range(n_bands):
                    s0 = band * band_out_pairs
                    last = i == n_img - 1 and band == n_bands - 1
                    ob = opool.tile([128, band_out_pairs, W], F16, tag="ob")
                    for sti, (k, off) in enumerate(st_last if last else st_norm):
                        ps = pspool.tile([128, k * PW], F32, tag="ps", name="ps")
                        f = k * PW - 1
                        for ph in range(2):
                            for kw in range(KS):
                                widx = ph * 3 + kw
                                a = (s0 + off + ph) * PW + kw
                                nc.tensor.matmul(
                                    ps[:, 0:f],
                                    w_sb[:, i * 6 + widx, :],
                                    xt[:, i, a : a + f],
                                    start=(widx == 0),
                                    stop=(widx == 5),
                                )
                        ps3 = ps.rearrange("p (t c) -> p t c", t=k)
                        nc.scalar.activation(
                            ob[:, off : off + k, :],
                            ps3[:, :, 0:W],
                            AF.Identity,
                            bias=bv_sb[:, i : i + 1],
                            scale=1.0,
                        )
                        if last:
                            # stream out in shrinking chunks on the (by now
                            # idle) sync HWDGE ring; the final 1-pair store
                            # issues from the scalar engine right behind its
                            # own ACT, dodging sync-ring issue serialization
                            for lo, hi, after in oc_last:
                                if after == sti:
                                    eng = (
                                        nc.scalar
                                        if after == len(st_last) - 1
                                        else nc.sync
                                    )
                                    eng.dma_start(
                                        out=out3[i, :, s0 + lo : s0 + hi, :],
                                        in_=ob[:, lo:hi, :],
                                    )
                    if not last:
                        # two half-band output DMAs (SWDGE ring, decoupled
                        # from the scalar ACT stream)
                        hb = band_out_pairs // 2
                        nc.gpsimd.dma_start(
                            out=out3[i, :, s0 : s0 + hb, :], in_=ob[:, 0:hb, :]
                        )
                        nc.gpsimd.dma_start(
                            out=out3[i, :, s0 + hb : s0 + band_out_pairs, :],
                            in_=ob[:, hb:band_out_pairs, :],
                        )

    nc.compile()
    return nc


_NC_CACHE = {}


def _get_nc():
    if "nc" not in _NC_CACHE:
        _NC_CACHE["nc"] = build_nc()
    return _NC_CACHE["nc"]


def _decode_weights(U, V, twopow, scale, biasq, bias):
    """Host-side decode of the tiny weight params -> fp16 lhsT tiles.

    Returns L (N, 6, 128, 128) fp16 and bn (N, 128) f32.
    lhsT tile widx = ph*3+kw, K index q = (row parity, cin), M = (out parity j,
    cout). Phase 1 reads rhs pair m (padded rows 2m, 2m+1), phase 2 pair m+1.
    """
    theta = np.einsum("ndk,bdk->nbd", U, V)  # (N, NB, D)
    sb = 1.0 / (1.0 + np.exp(-np.clip(theta, -10.0, 10.0)))
    integer = np.einsum("nbd,b->nd", sb, twopow)
    w = scale * (integer - biasq - 2.0**NB)  # (N, D)
    w = w.reshape(N, COUT, CIN, KS, KS)
    wq = np.ascontiguousarray(w.transpose(0, 2, 1, 3, 4))  # (n, ci, co, kh, kw)

    L = np.zeros((N, 6, 128, 128), np.float16)
    for kw in range(KS):
        # phase 1: (q0 -> j0): kh0, (q1 -> j0): kh1, (q1 -> j1): kh0
        L[:, kw, 0:64, 0:64] = wq[:, :, :, 0, kw]
        L[:, kw, 64:128, 0:64] = wq[:, :, :, 1, kw]
        L[:, kw, 64:128, 64:128] = wq[:, :, :, 0, kw]
        # phase 2: (q0 -> j0): kh2, (q0 -> j1): kh1, (q1 -> j1): kh2
        L[:, 3 + kw, 0:64, 0:64] = wq[:, :, :, 2, kw]
        L[:, 3 + kw, 0:64, 64:128] = wq[:, :, :, 1, kw]
        L[:, 3 + kw, 64:128, 64:128] = wq[:, :, :, 2, kw]

    bn = bias.reshape(N, COUT).astype(np.float32)
    bn = np.concatenate([bn, bn], axis=1)  # (N, 128)
    return L, bn


def _pack_x(xb):
    """(n, 64, 160, 160) f32 -> (n, 128, NPX) f16 parity layout, pads baked."""
    n = xb.shape[0]
    P = np.zeros((n, CIN, H + 2, H + 2), np.float16)
    P[:, :, 1 : H + 1, 1 : W + 1] = xb
    ev = P[:, :, 0 : H + 2 : 2, 0 : W + 1]  # (n, 64, 81, 161) padded rows 2s
    od = P[:, :, 1 : H + 2 : 2, 0 : W + 1]  # padded rows 2s+1
    arr = np.concatenate([ev, od], axis=1).reshape(n, 128, NPAIR * PW)
    out = np.zeros((n, 128, NPX), np.float16)
    out[:, :, 0 : NPAIR * PW] = arr
    return out


LAST_RESULT = None


def _ensure_ntff_hook():
    """The container's antenv package lacks axon_hooks; synthesize it so
    run_bass_kernel_spmd(trace=True) can register the NTFF profiler."""
    import sys
    import types

    if "antenv.axon_hooks" in sys.modules:
        return True
    try:
        import antenv
        from trn_agent_boot.trn_boot import _ntff_profile_via_ctypes

        hook = _ntff_profile_via_ctypes("/opt/axon/libaxon_pjrt.so")
        mod = types.ModuleType("antenv.axon_hooks")
        mod._hook = hook
        mod.get_axon_ntff_profile_hook = lambda: mod._hook
        mod.set_axon_ntff_profile_hook = lambda h: setattr(mod, "_hook", h)
        sys.modules["antenv.axon_hooks"] = mod
        antenv.axon_hooks = mod
        return hook is not None
    except Exception as e:  # degrade to untraced run
        print(f"ntff hook setup failed: {type(e).__name__}: {e}")
        return False


def kernel(x, U, V, twopow, scale, biasq, bias):
    from concourse.bass_utils import run_bass_kernel_spmd

    global LAST_RESULT
    x = np.asarray(x, np.float32)
    L, bn = _decode_weights(
        np.asarray(U, np.float32),
        np.asarray(V, np.float32),
        np.asarray(twopow, np.float32),
        np.asarray(scale, np.float32),
        np.asarray(biasq, np.float32),
        np.asarray(bias, np.float32),
    )

    in_maps = []
    for j in range(N_CORES):
        bs = [N_IMG * j + t for t in range(N_IMG)]
        ns = [b % N for b in bs]
        wlj = np.ascontiguousarray(
            L[ns].reshape(N_IMG * 6, 128, 128).transpose(1, 0, 2)
        )  # (128, n_img*6, 128)
        bvj = np.ascontiguousarray(bn[ns].T)  # (128, n_img)
        in_maps.append(
            {
                "x3": _pack_x(x[bs]),
                "wl": wlj,
                "bv": bvj,
            }
        )

    nc = _get_nc()
    trace = bool(os.environ.get("KERNEL_TRACE"))
    if trace:
        trace = _ensure_ntff_hook()
    tmpdir = os.environ.get("KERNEL_TRACE_DIR") or None
    res = run_bass_kernel_spmd(
        nc, in_maps, list(range(N_CORES)), trace=trace, tmpdir=tmpdir
    )
    LAST_RESULT = res

    out = np.empty((16, COUT, H, W), np.float32)
    for j in range(N_CORES):
        o3 = res.results[j]["out3"].astype(np.float32)  # (n_img, 128, 80, 160)
        for i in range(N_IMG):
            b = N_IMG * j + i
            out[b, :, 0::2, :] = o3[i, 0:64]
            out[b, :, 1::2, :] = o3[i, 64:128]
    return out


# revision 38
# speedup vs baseline: 1.0104x; 1.0022x over previous
"""Ensemble low-bit-decoded 3x3 conv2d, data-parallel over 8 TRN2 NeuronCores.

Problem (hardcoded): x (16, 64, 160, 160) f32. 4 ensemble members; image b uses
ensemble n = b % 4. Weights (64, 64, 3, 3) per ensemble are decoded from tiny
U/V/scale/biasq params:
    w = scale_n * (sigmoid(clip(U_n*V_0)) + 2*sigmoid(clip(U_n*V_1)) - biasq_n - 4)
then out[b] = conv2d(x[b], w_{b%4}, pad=1) + bias_{b%4}.

Sharding: core j gets images (2j, 2j+1); weights decoded host-side (tiny) and
shipped as ready fp16 lhsT tiles.

Kernel strategy per image (pure conv on device, memory-roofline oriented):
  Host packs the image into a parity SBUF layout, fp16, zero pads baked in:
  partition ci<64 = channel ci of even padded row r'=2s, 64+ci = odd r'=2s+1,
  free offset s*161 + c (col 0 = shared left/right pad). All device DMAs are
  therefore large fully-contiguous-per-partition transfers. A matmul with
  K=128 = (2 rows x 64 cin), M=128 = (2 out rows x 64 cout) covers up to 4 conv
  taps; 6 matmuls (2 row-phases x 3 kw shifts) accumulate a PSUM supertile of
  2-3 output row-pairs (F<=482), covering all 9 taps of the 3x3 stencil.
  Output is written fp16 in the same parity layout and unpacked on host.
"""

import os

import numpy as np

import concourse.bass as bass
import concourse.mybir as mybir
import concourse.tile as tile
from concourse import bacc

N = 4
CIN = 64
COUT = 64
KS = 3
NB = 2  # weight bits
H = 160
W = 160
N_CORES = 8
N_IMG = 2  # images per core

PW = W + 1  # pair stride in the shared-pad layout
NPAIR = (H + 2) // 2  # 81 padded row-pairs
NPX = NPAIR * PW + 1  # free elements per image per partition (13042)
OUT_PAIRS = H // 2  # 80

F32 = mybir.dt.float32
F16 = mybir.dt.float16


def build_nc(n_img=N_IMG, band_out_pairs=20, st_pairs=3, n_in_chunks=8):
    """Build the single-core Bass program (SPMD: all cores run this)."""
    assert OUT_PAIRS % band_out_pairs == 0
    n_bands = OUT_PAIRS // band_out_pairs

    nc = bacc.Bacc("TRN2", target_bir_lowering=False, num_swdge_queues=4)

    x3 = nc.dram_tensor("x3", (n_img, 128, NPX), F16, kind="ExternalInput")
    wl = nc.dram_tensor("wl", (128, n_img * 6, 128), F16, kind="ExternalInput")
    bv = nc.dram_tensor("bv", (128, n_img), F32, kind="ExternalInput")
    out3 = nc.dram_tensor(
        "out3", (n_img, 128, OUT_PAIRS, W), F16, kind="ExternalOutput"
    )

    AF = mybir.ActivationFunctionType

    # supertile split of each band; the last band tapers to a 1-pair final
    # supertile so the very last ACT + store are tiny
    def split(ks):
        o, offs = 0, []
        for k in ks:
            offs.append(o)
            o += k
        assert o == band_out_pairs
        return list(zip(ks, offs))

    sts = []
    rem = band_out_pairs
    while rem > 0:
        k = min(st_pairs, rem)
        sts.append(k)
        rem -= k
    st_norm = split(sts)
    st_last = split([3, 3, 3, 3, 3, 2, 2, 1])
    # last band's output chunks: (lo, hi, after-supertile-index); tapered so
    # the post-compute tail is just a 1-pair store
    oc_last = [(0, 9, 3), (9, 15, 4), (15, 17, 5), (17, 19, 6), (19, 20, 7)]

    with tile.TileContext(nc) as tc:
        with (
            tc.tile_pool(name="wts", bufs=1) as wpool,
            tc.tile_pool(name="xbuf", bufs=1) as xpool,
            tc.tile_pool(name="obuf", bufs=4) as opool,
            tc.tile_pool(name="psum", bufs=8, space="PSUM") as pspool,
        ):
            # HAM warmup: the PE clock sits at 1.2 GHz until ~3.4us of
            # sustained activity (a gap resets the free-running window). A
            # continuous train of small matmuls paces the PE from preamble
            # end (~7.5us) until the first real data lands (~10.5us): HAM
            # flips to full clock mid-train and the real stream starts warm.
            # The count is tuned to the data arrival — too few leaves a gap
            # (stream runs cold), too many delays the real stream.
            # (memset on gpsimd — its preamble ends first; the train start
            # itself is pinned by the tensor engine's preamble at ~7.4us)
            scr = wpool.tile([128, 512], F16, tag="scr")
            nc.gpsimd.memset(scr[:], 0.0)
            wps = pspool.tile([128, 256], F32, tag="ps", name="ps")
            for _ in range(15):
                nc.tensor.matmul(
                    wps[:], scr[:, 0:128], scr[:, 0:256], start=True, stop=True
                )

            w_sb = wpool.tile([128, n_img * 6, 128], F16, tag="w")
            bv_sb = wpool.tile([128, n_img], F32, tag="bv")
            xt = xpool.tile([128, n_img, NPX], F16, tag="x")

            # ALL loads ride ONE sync-ring FIFO in need-order: weights first,
            # then chunks in consumption order. The deep in-order supply
            # queue stays 1-2 chunks ahead of the matmul stream and cannot be
            # starved by another initiator racing ahead during the slow
            # post-idle HBM ramp (which capped aggregate rate at ~150 GB/s).
            bnds0 = [0, 4 * PW + 1, 9 * PW + 1, 14 * PW + 1, 21 * PW + 1] + [
                (31 + 10 * c) * PW + 1 for c in range(5)
            ] + [NPX]
            x0c = [(0, bnds0[c], bnds0[c + 1]) for c in range(len(bnds0) - 1)]
            cpairs = NPAIR // n_in_chunks
            bnds1 = [cpairs * c * PW for c in range(n_in_chunks)] + [NPX]
            x1c = [(1, bnds1[c], bnds1[c + 1]) for c in range(len(bnds1) - 1)]

            def xdma(eng, i, lo, hi):
                eng.dma_start(
                    out=xt[:, i, lo:hi], in_=x3[i, :, lo:hi]
                )

            # image-0 weights split phase-1/phase-2: supertile 0's first
            # three matmuls need only w0a + pairs 0-3. The first chunk rides
            # the scalar ring so its descriptor-gen overlaps w0a's on sync —
            # the two startup-critical loads land in parallel. Nothing else
            # early on scalar, so there is no flood to starve the chain.
            nc.sync.dma_start(out=w_sb[:, 0:3, :], in_=wl[:, 0:3, :])
            xdma(nc.scalar, *x0c[0])
            nc.scalar.dma_start(out=bv_sb[:], in_=bv[:, :])
            nc.sync.dma_start(out=w_sb[:, 3:6, :], in_=wl[:, 3:6, :])
            for c in x0c[1:]:
                xdma(nc.sync, *c)
            nc.sync.dma_start(out=w_sb[:, 6:12, :], in_=wl[:, 6:12, :])
            for c in x1c:
                xdma(nc.sync, *c)

            for i in range(n_img):
                for band in range(n_bands):
                    s0 = band * band_out_pairs
                    last = i == n_img - 1 and band == n_bands - 1
                    ob = opool.tile([128, band_out_pairs, W], F16, tag="ob")
                    for sti, (k, off) in enumerate(st_last if last else st_norm):
                        ps = pspool.tile([128, k * PW], F32, tag="ps", name="ps")
                        f = k * PW - 1
                        for ph in range(2):
                            for kw in range(KS):
                                widx = ph * 3 + kw
                                a = (s0 + off + ph) * PW + kw
                                nc.tensor.matmul(
                                    ps[:, 0:f],
                                    w_sb[:, i * 6 + widx, :],
                                    xt[:, i, a : a + f],
                                    start=(widx == 0),
                                    stop=(widx == 5),
                                )
                        ps3 = ps.rearrange("p (t c) -> p t c", t=k)
                        nc.scalar.activation(
                            ob[:, off : off + k, :],
                            ps3[:, :, 0:W],
                            AF.Identity,
                            bias=bv_sb[:, i : i + 1],
                            scale=1.0,
                        )
                        if last:
                            # stream out in shrinking chunks on the (by now
                            # idle) sync HWDGE ring; the final 1-pair store
                            # issues from the scalar engine right behind its
                            # own ACT, dodging sync-ring issue serialization
                            for lo, hi, after in oc_last:
                                if after == sti:
                                    eng = (
                                        nc.scalar
                                        if after == len(st_last) - 1
                                        else nc.sync
                                    )
                                    eng.dma_start(
                                        out=out3[i, :, s0 + lo : s0 + hi, :],
                                        in_=ob[:, lo:hi, :],
                                    )
                    if not last:
                        # two half-band output DMAs (SWDGE ring, decoupled
                        # from the scalar ACT stream)
                        hb = band_out_pairs // 2
                        nc.gpsimd.dma_start(
                            out=out3[i, :, s0 : s0 + hb, :], in_=ob[:, 0:hb, :]
                        )
                        nc.gpsimd.dma_start(
                            out=out3[i, :, s0 + hb : s0 + band_out_pairs, :],
                            in_=ob[:, hb:band_out_pairs, :],
                        )

    nc.compile()
    return nc


_NC_CACHE = {}


def _get_nc():
    if "nc" not in _NC_CACHE:
        _NC_CACHE["nc"] = build_nc()
    return _NC_CACHE["nc"]


def _decode_weights(U, V, twopow, scale, biasq, bias):
    """Host-side decode of the tiny weight params -> fp16 lhsT tiles.

    Returns L (N, 6, 128, 128) fp16 and bn (N, 128) f32.
    lhsT tile widx = ph*3+kw, K index q = (row parity, cin), M = (out parity j,
    cout). Phase 1 reads rhs pair m (padded rows 2m, 2m+1), phase 2 pair m+1.
    """
    theta = np.einsum("ndk,bdk->nbd", U, V)  # (N, NB, D)
    sb = 1.0 / (1.0 + np.exp(-np.clip(theta, -10.0, 10.0)))
    integer = np.einsum("nbd,b->nd", sb, twopow)
    w = scale * (integer - biasq - 2.0**NB)  # (N, D)
    w = w.reshape(N, COUT, CIN, KS, KS)
    wq = np.ascontiguousarray(w.transpose(0, 2, 1, 3, 4))  # (n, ci, co, kh, kw)

    L = np.zeros((N, 6, 128, 128), np.float16)
    for kw in range(KS):
        # phase 1: (q0 -> j0): kh0, (q1 -> j0): kh1, (q1 -> j1): kh0
        L[:, kw, 0:64, 0:64] = wq[:, :, :, 0, kw]
        L[:, kw, 64:128, 0:64] = wq[:, :, :, 1, kw]
        L[:, kw, 64:128, 64:128] = wq[:, :, :, 0, kw]
        # phase 2: (q0 -> j0): kh2, (q0 -> j1): kh1, (q1 -> j1): kh2
        L[:, 3 + kw, 0:64, 0:64] = wq[:, :, :, 2, kw]
        L[:, 3 + kw, 0:64, 64:128] = wq[:, :, :, 1, kw]
        L[:, 3 + kw, 64:128, 64:128] = wq[:, :, :, 2, kw]

    bn = bias.reshape(N, COUT).astype(np.float32)
    bn = np.concatenate([bn, bn], axis=1)  # (N, 128)
    return L, bn


def _pack_x(xb):
    """(n, 64, 160, 160) f32 -> (n, 128, NPX) f16 parity layout, pads baked."""
    n = xb.shape[0]
    P = np.zeros((n, CIN, H + 2, H + 2), np.float16)
    P[:, :, 1 : H + 1, 1 : W + 1] = xb
    ev = P[:, :, 0 : H + 2 : 2, 0 : W + 1]  # (n, 64, 81, 161) padded rows 2s
    od = P[:, :, 1 : H + 2 : 2, 0 : W + 1]  # padded rows 2s+1
    arr = np.concatenate([ev, od], axis=1).reshape(n, 128, NPAIR * PW)
    out = np.zeros((n, 128, NPX), np.float16)
    out[:, :, 0 : NPAIR * PW] = arr
    return out


LAST_RESULT = None


def _ensure_ntff_hook():
    """The container's antenv package lacks axon_hooks; synthesize it so
    run_bass_kernel_spmd(trace=True) can register the NTFF profiler."""
    import sys
    import types

    if "antenv.axon_hooks" in sys.modules:
        return True
    try:
        import antenv
        from trn_agent_boot.trn_boot import _ntff_profile_via_ctypes

        hook = _ntff_profile_via_ctypes("/opt/axon/libaxon_pjrt.so")
        mod = types.ModuleType("antenv.axon_hooks")
        mod._hook = hook
        mod.get_axon_ntff_profile_hook = lambda: mod._hook
        mod.set_axon_ntff_profile_hook = lambda h: setattr(mod, "_hook", h)
        sys.modules["antenv.axon_hooks"] = mod
        antenv.axon_hooks = mod
        return hook is not None
    except Exception as e:  # degrade to untraced run
        print(f"ntff hook setup failed: {type(e).__name__}: {e}")
        return False


def kernel(x, U, V, twopow, scale, biasq, bias):
    from concourse.bass_utils import run_bass_kernel_spmd

    global LAST_RESULT
    x = np.asarray(x, np.float32)
    L, bn = _decode_weights(
        np.asarray(U, np.float32),
        np.asarray(V, np.float32),
        np.asarray(twopow, np.float32),
        np.asarray(scale, np.float32),
        np.asarray(biasq, np.float32),
        np.asarray(bias, np.float32),
    )

    in_maps = []
    for j in range(N_CORES):
        bs = [N_IMG * j + t for t in range(N_IMG)]
        ns = [b % N for b in bs]
        wlj = np.ascontiguousarray(
            L[ns].reshape(N_IMG * 6, 128, 128).transpose(1, 0, 2)
        )  # (128, n_img*6, 128)
        bvj = np.ascontiguousarray(bn[ns].T)  # (128, n_img)
        in_maps.append(
            {
                "x3": _pack_x(x[bs]),
                "wl": wlj,
                "bv": bvj,
            }
        )

    nc = _get_nc()
    trace = bool(os.environ.get("KERNEL_TRACE"))
    if trace:
        trace = _ensure_ntff_hook()
    tmpdir = os.environ.get("KERNEL_TRACE_DIR") or None
    res = run_bass_kernel_spmd(
        nc, in_maps, list(range(N_CORES)), trace=trace, tmpdir=tmpdir
    )
    LAST_RESULT = res

    out = np.empty((16, COUT, H, W), np.float32)
    for j in range(N_CORES):
        o3 = res.results[j]["out3"].astype(np.float32)  # (n_img, 128, 80, 160)
        for i in range(N_IMG):
            b = N_IMG * j + i
            out[b, :, 0::2, :] = o3[i, 0:64]
            out[b, :, 1::2, :] = o3[i, 64:128]
    return out
